# revision 56
# baseline (speedup 1.0000x reference)
"""BandhaAttention Trainium2 kernel (v3 — natural-layout AV, combined proj).

Sharding: 8 cores = 2 (batch) x 4 (head groups of 4 heads).
Per core, heads are processed as 2 pairs of 2 heads. Scores are computed
transposed (keys on partitions), AV runs in natural layout: av[q, 65]
accumulates over key tiles with exp tiles as stationary weights,
streaming v plus a ones column (softmax sums for free). Normalization is
a per-partition scalar divide on DVE, the [q, c] -> [c, q] transpose for
the out-projection is done by the DMA xbar (SBUF->SBUF), and the
out-projection contracts both pairs into one bf16 partial output per
core (host sums 8 partials). Pair 0 walks stripes 0..3 while pair 1
walks 1,2,3,0 one super-stripe behind, so the exp workload on ACT
spreads evenly and the final stripe (pair 1, stripe 0) is tiny, keeping
the drain short. qkv projection for pair 1, v projection, and the
out-projections run as PE filler inside the attention stripes.
"""

import os
import sys

import numpy as np

for p in ("/opt/trn_rl_repo", "/opt/trn_rl_repo/concourse"):
    if p not in sys.path and os.path.isdir(p):
        sys.path.insert(0, p)

import ml_dtypes

import concourse.bacc as bacc
import concourse.mybir as mybir
from concourse.bass_utils import run_bass_kernel_spmd
from concourse.tile import TileContext

BF16 = mybir.dt.bfloat16
F32 = mybir.dt.float32
AF = mybir.ActivationFunctionType
ALU = mybir.AluOpType

T = 2048
D = 1024
HD = 64
KT = 8          # contraction chunks of 128 for the qkv projection

TALA = [5, 6, 7, 8]

LAST = None  # last BassKernelResults (for profiling from test.py)


def build_nc(reps=1):
    nc = bacc.Bacc("TRN2", target_bir_lowering=False)
    xt_d = nc.dram_tensor("xt", [D, T], BF16, kind="ExternalInput")
    wqk_d = nc.dram_tensor("wqk", [D, 512], BF16, kind="ExternalInput")
    wv_d = nc.dram_tensor("wv", [D, 256], BF16, kind="ExternalInput")
    wout_d = nc.dram_tensor("wout", [128, 2 * D], BF16, kind="ExternalInput")
    gate_d = nc.dram_tensor("gate", [256, T], BF16, kind="ExternalInput")
    tri_d = nc.dram_tensor("tri", [128, 128], BF16, kind="ExternalInput")
    eye_d = nc.dram_tensor("eye", [128, 128], BF16, kind="ExternalInput")
    out_d = nc.dram_tensor("out", [T, D], BF16, kind="ExternalOutput")

    with TileContext(nc) as tc:
      for rep in range(reps):
        with (
            tc.tile_pool(name=f"pers{rep}", bufs=1) as pers,
            tc.tile_pool(name=f"pexp{rep}", bufs=40) as pexp,
            tc.tile_pool(name=f"pao{rep}", bufs=32) as pao,
            tc.tile_pool(name=f"pstg{rep}", bufs=4) as pstg,
            tc.tile_pool(name=f"prc{rep}", bufs=4) as prc,
            tc.tile_pool(name=f"psq{rep}", bufs=2, space="PSUM") as psq,
            tc.tile_pool(name=f"pst{rep}", bufs=2, space="PSUM") as pst,
            tc.tile_pool(name=f"pav{rep}", bufs=2, space="PSUM") as pav,
        ):
            # ---- persistent SBUF tiles ----
            xt_sb = pers.tile([128, KT, T], BF16, tag="xt", name="xt_sb")
            wqk_sb = pers.tile([128, KT, 512], BF16, tag="wqk", name="wqk_sb")
            wv_sb = pers.tile([128, KT, 256], BF16, tag="wv", name="wv_sb")
            wout_sb = pers.tile([128, 2, D], BF16, tag="wout", name="wout_sb")
            gate_sb = pers.tile([128, 2, T], BF16, tag="gate", name="gate_sb")
            tri = pers.tile([128, 128], BF16, tag="tri", name="tri")
            eye = pers.tile([128, 128], BF16, tag="eye", name="eye")
            qp = [pers.tile([128, T], BF16, tag=f"qp{p}", name=f"qp{p}")
                  for p in range(2)]
            kp = [pers.tile([128, T], BF16, tag=f"kp{p}", name=f"kp{p}")
                  for p in range(2)]
            v8 = pers.tile([128, 16, 4, 65], BF16, tag="v8", name="v8")

            # ---- DMA preamble, ordered for earliest first matmul ----
            nc.sync.dma_start(wqk_sb[:, 0, :], wqk_d[0:128, :])
            nc.sync.dma_start(xt_sb[:, 0, 0:512], xt_d[0:128, 0:512])
            nc.sync.dma_start(wqk_sb[:, 1, :], wqk_d[128:256, :])
            nc.sync.dma_start(xt_sb[:, 1, 0:512], xt_d[128:256, 0:512])
            nc.sync.dma_start(
                wqk_sb[:, 2:4, :],
                wqk_d[256:512, :].rearrange("(a p) c -> p a c", p=128))
            for kc in range(2, 4):
                nc.sync.dma_start(
                    xt_sb[:, kc, 0:512],
                    xt_d[kc * 128:(kc + 1) * 128, 0:512])
            nc.sync.dma_start(
                wqk_sb[:, 4:KT, :],
                wqk_d[512:D, :].rearrange("(a p) c -> p a c", p=128))
            for kc in range(4, 6):
                nc.sync.dma_start(
                    xt_sb[:, kc, 0:512],
                    xt_d[kc * 128:(kc + 1) * 128, 0:512])
            nc.sync.dma_start(gate_sb[:, 0, 0:512], gate_d[0:128, 0:512])
            for kc in range(6, KT):
                nc.sync.dma_start(
                    xt_sb[:, kc, 0:512],
                    xt_d[kc * 128:(kc + 1) * 128, 0:512])
            nc.sync.dma_start(tri, tri_d[:, :])
            nc.sync.dma_start(eye, eye_d[:, :])
            nc.sync.dma_start(
                wv_sb, wv_d[:, :].rearrange("(a p) c -> p a c", p=128))
            nc.sync.dma_start(gate_sb[:, 1, 0:512], gate_d[128:256, 0:512])
            nc.sync.dma_start(gate_sb[:, 0, 512:T], gate_d[0:128, 512:T])
            nc.sync.dma_start(gate_sb[:, 1, 512:T], gate_d[128:256, 512:T])
            for kc in range(KT):
                nc.sync.dma_start(
                    xt_sb[:, kc, 512:1024],
                    xt_d[kc * 128:(kc + 1) * 128, 512:1024])
            for kc in range(KT):
                nc.sync.dma_start(
                    xt_sb[:, kc, 1024:T],
                    xt_d[kc * 128:(kc + 1) * 128, 1024:T])
            nc.sync.dma_start(
                wout_sb.rearrange("p a c -> p (a c)"), wout_d[:, :])

            # v8 ones columns (softmax denominators via matmul)
            nc.gpsimd.memset(v8[:, :, :, 64:65], 1.0)

            # ---- engine work units ----

            qk_ps = {}

            def do_qk(m, n, part):
                """qkv projection m-tile (0:q-p0 1:q-p1 2:k-p0 3:k-p1),
                column chunk n (512 queries); split in two for smoother
                interleaving with score pieces."""
                if part == 0:
                    qk_ps[(m, n)] = psq.tile([128, 512], F32, tag="psq",
                                             name="ps_qk")
                ps = qk_ps[(m, n)]
                for kc in range(4 * part, 4 * part + 4):
                    nc.tensor.matmul(
                        ps,
                        lhsT=wqk_sb[:, kc, m * 128:(m + 1) * 128],
                        rhs=xt_sb[:, kc, n * 512:(n + 1) * 512],
                        start=(kc == 0), stop=(kc == KT - 1),
                    )
                if part == 0:
                    return
                p = m % 2
                dst = (qp if m < 2 else kp)[p]
                if m < 2:  # gate the queries while evacuating
                    nc.vector.tensor_mul(
                        dst[:, n * 512:(n + 1) * 512], ps,
                        gate_sb[:, p, n * 512:(n + 1) * 512])
                else:
                    nc.vector.tensor_copy(dst[:, n * 512:(n + 1) * 512], ps)

            v_ps = {}

            def do_v(t, part):
                """v natural projection for key tile t -> v8[:, t, :, 0:64]."""
                if part == 0:
                    v_ps[t] = psq.tile([128, 256], F32, tag="psq",
                                       name="ps_v")
                ps = v_ps[t]
                for kc in range(4 * part, 4 * part + 4):
                    nc.tensor.matmul(
                        ps,
                        lhsT=xt_sb[:, kc, t * 128:(t + 1) * 128],
                        rhs=wv_sb[:, kc, :],
                        start=(kc == 0), stop=(kc == KT - 1),
                    )
                if part == 1:
                    nc.vector.tensor_copy(
                        v8[:, t, :, 0:64],
                        ps.rearrange("p (h c) -> p h c", c=64))

            expt = {}  # (pair, i, j) -> [128, 2, 512] bf16 tile

            def do_st(p, i, j, mask_eng="pool"):
                """scores^T piece: key tile i, stripe j, both heads of pair
                p, exp'd into expt[(p, i, j)][:, :, r:512]."""
                r = max(0, 128 * i - 512 * j)
                w = 512 - r
                a = 512 * j + r
                st = pst.tile([128, 1024], F32, tag="st", name="st_ps")
                stv = st.rearrange("p (h c) -> p h c", c=512)
                for hh in range(2):
                    lo = hh * 64
                    nc.tensor.matmul(
                        stv[:, hh, r:512],
                        lhsT=kp[p][lo:lo + 64, i * 128:(i + 1) * 128],
                        rhs=qp[p][lo:lo + 64, a:a + w],
                        start=True, stop=True,
                    )
                e = pexp.tile([128, 2, 512], BF16, tag="exp", name="exp_sb")
                expt[(p, i, j)] = e
                nc.scalar.activation(
                    e[:, :, r:512], stv[:, :, r:512], AF.Exp, scale=0.125)
                if i >= 4 * j:  # diagonal piece: causal band mask
                    eng = nc.vector if mask_eng == "dve" else nc.gpsimd
                    for hh in range(2):
                        eng.tensor_mul(
                            e[:, hh, r:r + 128], e[:, hh, r:r + 128], tri)

            aoN = {}

            def do_chain(p, hh, b):
                """natural AV for head hh of pair p, query block b: accumulate
                av[q, 65] over key tiles 0..b, then normalize into aoN."""
                j = b // 4
                av = pav.tile([128, 65], F32, tag="av", name="av_ps")
                for i in range(b + 1):
                    e = expt[(p, i, j)]
                    qq = 128 * (b - 4 * j)
                    nc.tensor.matmul(
                        av,
                        lhsT=e[:, hh, qq:qq + 128],
                        rhs=v8[:, i, 2 * p + hh, :],
                        start=(i == 0), stop=(i == b),
                    )
                if (p, b) not in aoN:
                    aoN[(p, b)] = pao.tile([128, 128], BF16, tag="aoN",
                                           name="aoN_sb")
                rc = prc.tile([128, 1], F32, tag="rc", name="rc_sb")
                nc.vector.reciprocal(rc, av[:, 64:65])
                nc.vector.tensor_scalar(
                    aoN[(p, b)][:, hh * 64:hh * 64 + 64],
                    av[:, 0:64], rc, None, ALU.mult)

            aoT = {}

            def do_transpose(p, b, via="dma"):
                aoT[(p, b)] = pao.tile([128, 128], BF16, tag="aoT",
                                       name="aoT_sb")
                if via == "pe":
                    # tail blocks: PE transpose + ACT evac beats the ~2.3us
                    # DMA xbar latency; the pst ring is idle by then
                    tp = pst.tile([128, 128], BF16, tag="st", name="tp_ps")
                    nc.tensor.transpose(tp, aoN[(p, b)], eye)
                    nc.scalar.copy(aoT[(p, b)], tp)
                else:
                    nc.sync.dma_start(aoT[(p, b)], aoN[(p, b)], transpose=True)

            stg_tiles = {}

            def do_proj(t, n, evac="dve", split=False):
                """out-projection for query block t, column half n: contract
                both pairs (256 channels) into one po, evacuate bf16, DMA
                the finished output out."""
                po = psq.tile([128, 512], F32, tag="psq", name="po_ps")
                for p in range(2):
                    nc.tensor.matmul(
                        po,
                        lhsT=aoT[(p, t)],
                        rhs=wout_sb[:, p, n * 512:(n + 1) * 512],
                        start=(p == 0), stop=(p == 1),
                    )
                if n == 0:
                    stg_tiles[t] = pstg.tile([128, D], BF16, tag="stg",
                                             name="stg_sb")
                stg = stg_tiles[t]
                if evac == "act":  # tail blocks: ACT is done with exp there
                    nc.scalar.copy(stg[:, n * 512:(n + 1) * 512], po)
                else:
                    nc.vector.tensor_copy(stg[:, n * 512:(n + 1) * 512], po)
                if split:  # tail: per-half DMA so the drain overlaps
                    nc.sync.dma_start(
                        out_d[t * 128:(t + 1) * 128, n * 512:(n + 1) * 512],
                        stg[:, n * 512:(n + 1) * 512])
                elif n == 1:
                    nc.sync.dma_start(out_d[t * 128:(t + 1) * 128, :], stg)

            # ---- schedule ----

            def chain_unit(p, b, via="dma"):
                def u():
                    do_chain(p, 0, b)
                    do_chain(p, 1, b)
                    do_transpose(p, b, via)
                # chains stream v tiles 0..b as matmul rhs
                return (u, tuple(f"v{i}" for i in range(b + 1)))

            def stripe_units(p, j, i0=0, i1=None, chains=True, via="dma",
                             mask_eng="pool"):
                units = []
                for i in range(i0, 4 * j + 4 if i1 is None else i1):
                    units.append(
                        (lambda p=p, i=i, j=j, me=mask_eng:
                         do_st(p, i, j, me),
                         (f"m{2 + p}n{i // 4}", f"m{p}n{j}")))
                    if chains and i >= 4 * j:
                        units.append(chain_unit(p, i, via))
                return units

            def merge(u1, u2):
                out = []
                i1 = i2 = 0
                n1, n2 = len(u1), len(u2)
                while i1 < n1 or i2 < n2:
                    if i2 >= n2 or (i1 < n1 and i1 * n2 <= i2 * n1):
                        out.append(u1[i1])
                        i1 += 1
                    else:
                        out.append(u2[i2])
                        i2 += 1
                return out

            emitted = set()

            def interleave(primary, filler):
                """primary: [(unit, needs)]; filler: [(name, unit, deps)].
                Emits fillers proportionally, but pulls a named filler (and
                its own deps, recursively) forward whenever an upcoming
                primary depends on it, so every engine stream stays
                dependency-ordered (engines execute in order; a consumer
                emitted before its producer would deadlock real hardware)."""
                fi = 0
                index = {f[0]: f for f in filler}

                def fire_name(nm):
                    if nm in emitted:
                        return
                    assert nm in index, f"dependency {nm} not in filler list"
                    _, u, deps = index[nm]
                    for d in deps:
                        fire_name(d)
                    emitted.add(nm)
                    u()

                for k, (pu, needs) in enumerate(primary):
                    for nm in needs:
                        if nm not in emitted:
                            fire_name(nm)
                    pu()
                    target = ((k + 1) * len(filler)) // len(primary)
                    while fi < target:
                        fire_name(filler[fi][0])
                        fi += 1
                while fi < len(filler):
                    fire_name(filler[fi][0])
                    fi += 1

            def qk_u(m, n):
                """two filler units: matmul halves; name completes at part 1"""
                return [(f"m{m}n{n}_a", lambda: do_qk(m, n, 0), ()),
                        (f"m{m}n{n}", lambda: do_qk(m, n, 1),
                         (f"m{m}n{n}_a",))]

            def v_u(t):
                return [(f"v{t}_a", lambda: do_v(t, 0), ()),
                        (f"v{t}", lambda: do_v(t, 1), (f"v{t}_a",))]

            def proj_u(t, n, evac="dve"):
                return [(f"proj{t}_{n}", lambda: do_proj(t, n, evac), ())]

            # preamble: pair-0 q/k first column chunks, kc-interleaved so
            # both accumulators advance as each xt/wqk chunk lands
            ps_a = psq.tile([128, 512], F32, tag="psq", name="ps_qk")
            ps_b = psq.tile([128, 512], F32, tag="psq", name="ps_qk")
            for kc in range(KT):
                for m, ps in ((0, ps_a), (2, ps_b)):
                    nc.tensor.matmul(
                        ps,
                        lhsT=wqk_sb[:, kc, m * 128:(m + 1) * 128],
                        rhs=xt_sb[:, kc, 0:512],
                        start=(kc == 0), stop=(kc == KT - 1),
                    )
            nc.vector.tensor_mul(qp[0][:, 0:512], ps_a, gate_sb[:, 0, 0:512])
            nc.vector.tensor_copy(kp[0][:, 0:512], ps_b)
            emitted.update({"m0n0", "m2n0"})

            # Engines execute their streams in order, so every unit is
            # emitted after everything it depends on: v tiles land before the
            # chains that read them, q/k chunks a super-stripe before their
            # stripes, proj after both pairs' transposes. Pair 1 runs one
            # super-stripe behind pair 0; its stripe-3 scores/exp are
            # pre-computed inside ss3 so the final chain phase (ss4) has no
            # exp dependency and the drain stays short.
            def flat(*groups):
                return [f for g in groups for f in g]

            # ss0: p0 s0
            interleave(
                stripe_units(0, 0),
                flat(v_u(0), qk_u(3, 0), v_u(1), qk_u(2, 1), v_u(2),
                     qk_u(0, 1), v_u(3), qk_u(1, 0)),
            )
            # ss1: p0 s1 + p1 s0
            interleave(
                merge(stripe_units(0, 1), stripe_units(1, 0)),
                flat(qk_u(3, 1), v_u(4), qk_u(1, 1), v_u(5), qk_u(2, 2),
                     v_u(6), qk_u(0, 2), v_u(7)),
            )
            # ss2: p0 s2 + p1 s1 | combined proj t0-3 possible now
            interleave(
                merge(stripe_units(0, 2), stripe_units(1, 1)),
                flat(qk_u(3, 2), v_u(8), qk_u(1, 2), v_u(9), qk_u(2, 3),
                     v_u(10), qk_u(0, 3), v_u(11), qk_u(1, 3), qk_u(3, 3)),
            )
            # ss3: p0 s3 + p1 s2 + first part of p1 s3 scores | proj t0-7
            interleave(
                merge(merge(stripe_units(0, 3), stripe_units(1, 2)),
                      stripe_units(1, 3, i1=4)),
                flat(v_u(12), v_u(13), v_u(14), v_u(15),
                     *[proj_u(t, n) for t in range(0, 8) for n in range(2)]),
            )
            # ss4: rest of p1 s3 + its chains (PE transpose, DVE masks)
            interleave(
                stripe_units(1, 3, i0=4, via="pe", mask_eng="dve"),
                flat(*[proj_u(t, n) for t in range(8, 12) for n in range(2)]),
            )
            # tail: proj t12-15, evacuation split across DVE and ACT,
            # per-half output DMA
            for t in range(12, 16):
                for n in range(2):
                    do_proj(t, n, "act" if n else "dve", split=True)
    nc.compile()
    return nc


def _prep_inputs(x, w_qkv, w_out, bandha_gate):
    bf = ml_dtypes.bfloat16
    t = np.arange(T)
    gate_full = np.empty((16, T), np.float64)
    for h in range(16):
        cyc = TALA[h % len(TALA)]
        gate_full[h] = 1.0 / (
            1.0 + np.exp(-bandha_gate[h, t % cyc].astype(np.float64)))
    tri = (np.arange(128)[None, :] >= np.arange(128)[:, None]).astype(bf)
    eye = np.eye(128, dtype=np.float32).astype(bf)

    in_maps = []
    for c in range(8):
        b, g = c // 4, c % 4
        xt = np.ascontiguousarray(x[b].T).astype(bf)
        # m-tiles: q-pair0, q-pair1, k-pair0, k-pair1 (128 channels each)
        q0 = w_qkv[:, g * 256:g * 256 + 128]
        q1 = w_qkv[:, g * 256 + 128:g * 256 + 256]
        k0 = w_qkv[:, D + g * 256:D + g * 256 + 128]
        k1 = w_qkv[:, D + g * 256 + 128:D + g * 256 + 256]
        wqk = np.concatenate([q0, q1, k0, k1], axis=1).astype(bf)
        wv = np.ascontiguousarray(
            w_qkv[:, 2 * D + g * 256:2 * D + (g + 1) * 256]).astype(bf)
        # wout rows: within-pair channel, cols: (pair, d)
        wo = np.stack([w_out[g * 256:g * 256 + 128, :],
                       w_out[g * 256 + 128:g * 256 + 256, :]], axis=1)
        wo = np.ascontiguousarray(wo.reshape(128, 2 * D)).astype(bf)
        # gate rows: (pair, within-pair channel); pair p = heads {2p, 2p+1}
        gb = np.repeat(gate_full[4 * g:4 * g + 4].astype(np.float32),
                       HD, axis=0).astype(bf)
        in_maps.append({"xt": xt, "wqk": wqk, "wv": wv, "wout": wo,
                        "gate": np.ascontiguousarray(gb), "tri": tri,
                        "eye": eye})
    return in_maps


def kernel(**inputs):
    global LAST
    x = np.asarray(inputs["x"], np.float32)
    w_qkv = np.asarray(inputs["w_qkv"], np.float32)
    w_out = np.asarray(inputs["w_out"], np.float32)
    bandha_gate = np.asarray(inputs["bandha_gate"], np.float32)

    in_maps = _prep_inputs(x, w_qkv, w_out, bandha_gate)
    nc = build_nc()
    res = run_bass_kernel_spmd(
        nc, in_maps, core_ids=list(range(8)),
        trace=os.environ.get("BANDHA_TRACE") == "1",
    )
    LAST = res
    full = np.empty((2, T, D), np.float32)
    for b in range(2):
        acc = np.zeros((T, D), np.float32)
        for g in range(4):
            acc += np.asarray(res.results[4 * b + g]["out"],
                              dtype=np.float32)
        full[b] = acc
    return full


# revision 66
# speedup vs baseline: 1.0146x; 1.0146x over previous
"""BandhaAttention Trainium2 kernel (v3 — natural-layout AV, combined proj).

Sharding: 8 cores = 2 (batch) x 4 (head groups of 4 heads).
Per core, heads are processed as 2 pairs of 2 heads. Scores are computed
transposed (keys on partitions), AV runs in natural layout: av[q, 65]
accumulates over key tiles with exp tiles as stationary weights,
streaming v plus a ones column (softmax sums for free). Normalization is
a per-partition scalar divide on DVE, the [q, c] -> [c, q] transpose for
the out-projection is done by the DMA xbar (SBUF->SBUF), and the
out-projection contracts both pairs into one bf16 partial output per
core (host sums 8 partials). Pair 0 walks stripes 0..3 while pair 1
walks 1,2,3,0 one super-stripe behind, so the exp workload on ACT
spreads evenly and the final stripe (pair 1, stripe 0) is tiny, keeping
the drain short. qkv projection for pair 1, v projection, and the
out-projections run as PE filler inside the attention stripes.
"""

import os
import sys

import numpy as np

for p in ("/opt/trn_rl_repo", "/opt/trn_rl_repo/concourse"):
    if p not in sys.path and os.path.isdir(p):
        sys.path.insert(0, p)

import ml_dtypes

import concourse.bacc as bacc
import concourse.mybir as mybir
from concourse.bass_utils import run_bass_kernel_spmd
from concourse.tile import TileContext

BF16 = mybir.dt.bfloat16
F32 = mybir.dt.float32
AF = mybir.ActivationFunctionType
ALU = mybir.AluOpType

T = 2048
D = 1024
HD = 64
KT = 8          # contraction chunks of 128 for the qkv projection

TALA = [5, 6, 7, 8]

LAST = None  # last BassKernelResults (for profiling from test.py)


def build_nc(reps=1):
    nc = bacc.Bacc("TRN2", target_bir_lowering=False)
    xt_d = nc.dram_tensor("xt", [D, T], BF16, kind="ExternalInput")
    wqk_d = nc.dram_tensor("wqk", [D, 512], BF16, kind="ExternalInput")
    wv_d = nc.dram_tensor("wv", [D, 256], BF16, kind="ExternalInput")
    wout_d = nc.dram_tensor("wout", [128, 2 * D], BF16, kind="ExternalInput")
    gate_d = nc.dram_tensor("gate", [256, T], BF16, kind="ExternalInput")
    tri_d = nc.dram_tensor("tri", [128, 128], BF16, kind="ExternalInput")
    eye_d = nc.dram_tensor("eye", [128, 128], BF16, kind="ExternalInput")
    out_d = nc.dram_tensor("out", [T, D], BF16, kind="ExternalOutput")

    with TileContext(nc) as tc:
      for rep in range(reps):
        with (
            tc.tile_pool(name=f"pers{rep}", bufs=1) as pers,
            tc.tile_pool(name=f"pexp{rep}", bufs=40) as pexp,
            tc.tile_pool(name=f"pao{rep}", bufs=32) as pao,
            tc.tile_pool(name=f"pstg{rep}", bufs=4) as pstg,
            tc.tile_pool(name=f"prc{rep}", bufs=4) as prc,
            tc.tile_pool(name=f"psq{rep}", bufs=2, space="PSUM") as psq,
            tc.tile_pool(name=f"pst{rep}", bufs=2, space="PSUM") as pst,
            tc.tile_pool(name=f"pav{rep}", bufs=2, space="PSUM") as pav,
        ):
            # ---- persistent SBUF tiles ----
            xt_sb = pers.tile([128, KT, T], BF16, tag="xt", name="xt_sb")
            wqk_sb = pers.tile([128, KT, 512], BF16, tag="wqk", name="wqk_sb")
            wv_sb = pers.tile([128, KT, 256], BF16, tag="wv", name="wv_sb")
            wout_sb = pers.tile([128, 2, D], BF16, tag="wout", name="wout_sb")
            gate_sb = pers.tile([128, 2, T], BF16, tag="gate", name="gate_sb")
            tri = pers.tile([128, 128], BF16, tag="tri", name="tri")
            eye = pers.tile([128, 128], BF16, tag="eye", name="eye")
            qp = [pers.tile([128, T], BF16, tag=f"qp{p}", name=f"qp{p}")
                  for p in range(2)]
            kp = [pers.tile([128, T], BF16, tag=f"kp{p}", name=f"kp{p}")
                  for p in range(2)]
            v8 = pers.tile([128, 16, 4, 65], BF16, tag="v8", name="v8")

            # ---- DMA preamble, ordered for earliest first matmul ----
            nc.sync.dma_start(
                wqk_sb[:, 0:2, :],
                wqk_d[0:256, :].rearrange("(a p) c -> p a c", p=128))
            for kc in range(2):
                nc.sync.dma_start(
                    xt_sb[:, kc, 0:512],
                    xt_d[kc * 128:(kc + 1) * 128, 0:512])
            nc.sync.dma_start(
                wqk_sb[:, 2:4, :],
                wqk_d[256:512, :].rearrange("(a p) c -> p a c", p=128))
            for kc in range(2, 4):
                nc.sync.dma_start(
                    xt_sb[:, kc, 0:512],
                    xt_d[kc * 128:(kc + 1) * 128, 0:512])
            nc.sync.dma_start(
                wqk_sb[:, 4:KT, :],
                wqk_d[512:D, :].rearrange("(a p) c -> p a c", p=128))
            for kc in range(4, 6):
                nc.sync.dma_start(
                    xt_sb[:, kc, 0:512],
                    xt_d[kc * 128:(kc + 1) * 128, 0:512])
            nc.sync.dma_start(gate_sb[:, 0, 0:512], gate_d[0:128, 0:512])
            for kc in range(6, KT):
                nc.sync.dma_start(
                    xt_sb[:, kc, 0:512],
                    xt_d[kc * 128:(kc + 1) * 128, 0:512])
            nc.sync.dma_start(tri, tri_d[:, :])
            nc.sync.dma_start(eye, eye_d[:, :])
            nc.sync.dma_start(
                wv_sb, wv_d[:, :].rearrange("(a p) c -> p a c", p=128))
            nc.sync.dma_start(gate_sb[:, 1, 0:512], gate_d[128:256, 0:512])
            nc.sync.dma_start(gate_sb[:, 0, 512:T], gate_d[0:128, 512:T])
            nc.sync.dma_start(gate_sb[:, 1, 512:T], gate_d[128:256, 512:T])
            for kc in range(KT):
                nc.sync.dma_start(
                    xt_sb[:, kc, 512:1024],
                    xt_d[kc * 128:(kc + 1) * 128, 512:1024])
            for kc in range(KT):
                nc.sync.dma_start(
                    xt_sb[:, kc, 1024:T],
                    xt_d[kc * 128:(kc + 1) * 128, 1024:T])
            nc.sync.dma_start(
                wout_sb.rearrange("p a c -> p (a c)"), wout_d[:, :])

            # v8 ones columns (softmax denominators via matmul)
            nc.gpsimd.memset(v8[:, :, :, 64:65], 1.0)

            # ---- engine work units ----

            qk_ps = {}

            def do_qk(m, n, part):
                """qkv projection m-tile (0:q-p0 1:q-p1 2:k-p0 3:k-p1),
                column chunk n (512 queries); split in two for smoother
                interleaving with score pieces."""
                if part == 0:
                    qk_ps[(m, n)] = psq.tile([128, 512], F32, tag="psq",
                                             name="ps_qk")
                ps = qk_ps[(m, n)]
                for kc in range(4 * part, 4 * part + 4):
                    nc.tensor.matmul(
                        ps,
                        lhsT=wqk_sb[:, kc, m * 128:(m + 1) * 128],
                        rhs=xt_sb[:, kc, n * 512:(n + 1) * 512],
                        start=(kc == 0), stop=(kc == KT - 1),
                    )
                if part == 0:
                    return
                p = m % 2
                dst = (qp if m < 2 else kp)[p]
                if m < 2:  # gate the queries while evacuating
                    nc.vector.tensor_mul(
                        dst[:, n * 512:(n + 1) * 512], ps,
                        gate_sb[:, p, n * 512:(n + 1) * 512])
                else:
                    nc.vector.tensor_copy(dst[:, n * 512:(n + 1) * 512], ps)

            v_ps = {}

            def do_v(t, part):
                """v natural projection for key tile t -> v8[:, t, :, 0:64]."""
                if part == 0:
                    v_ps[t] = psq.tile([128, 256], F32, tag="psq",
                                       name="ps_v")
                ps = v_ps[t]
                for kc in range(4 * part, 4 * part + 4):
                    nc.tensor.matmul(
                        ps,
                        lhsT=xt_sb[:, kc, t * 128:(t + 1) * 128],
                        rhs=wv_sb[:, kc, :],
                        start=(kc == 0), stop=(kc == KT - 1),
                    )
                if part == 1:
                    nc.vector.tensor_copy(
                        v8[:, t, :, 0:64],
                        ps.rearrange("p (h c) -> p h c", c=64))

            expt = {}  # (pair, i, j) -> [128, 2, 512] bf16 tile

            def do_st(p, i, j, mask_eng="pool"):
                """scores^T piece: key tile i, stripe j, both heads of pair
                p, exp'd into expt[(p, i, j)][:, :, r:512]."""
                r = max(0, 128 * i - 512 * j)
                w = 512 - r
                a = 512 * j + r
                st = pst.tile([128, 1024], F32, tag="st", name="st_ps")
                stv = st.rearrange("p (h c) -> p h c", c=512)
                for hh in range(2):
                    lo = hh * 64
                    nc.tensor.matmul(
                        stv[:, hh, r:512],
                        lhsT=kp[p][lo:lo + 64, i * 128:(i + 1) * 128],
                        rhs=qp[p][lo:lo + 64, a:a + w],
                        start=True, stop=True,
                    )
                e = pexp.tile([128, 2, 512], BF16, tag="exp", name="exp_sb")
                expt[(p, i, j)] = e
                nc.scalar.activation(
                    e[:, :, r:512], stv[:, :, r:512], AF.Exp, scale=0.125)
                if i >= 4 * j:  # diagonal piece: causal band mask
                    eng = nc.vector if mask_eng == "dve" else nc.gpsimd
                    for hh in range(2):
                        eng.tensor_mul(
                            e[:, hh, r:r + 128], e[:, hh, r:r + 128], tri)

            aoN = {}

            def do_chain(p, hh, b):
                """natural AV for head hh of pair p, query block b: accumulate
                av[q, 65] over key tiles 0..b, then normalize into aoN."""
                j = b // 4
                av = pav.tile([128, 65], F32, tag="av", name="av_ps")
                for i in range(b + 1):
                    e = expt[(p, i, j)]
                    qq = 128 * (b - 4 * j)
                    nc.tensor.matmul(
                        av,
                        lhsT=e[:, hh, qq:qq + 128],
                        rhs=v8[:, i, 2 * p + hh, :],
                        start=(i == 0), stop=(i == b),
                    )
                if (p, b) not in aoN:
                    aoN[(p, b)] = pao.tile([128, 128], BF16, tag="aoN",
                                           name="aoN_sb")
                rc = prc.tile([128, 1], F32, tag="rc", name="rc_sb")
                nc.vector.reciprocal(rc, av[:, 64:65])
                nc.vector.tensor_scalar(
                    aoN[(p, b)][:, hh * 64:hh * 64 + 64],
                    av[:, 0:64], rc, None, ALU.mult)

            aoT = {}

            def do_transpose(p, b, via="dma"):
                aoT[(p, b)] = pao.tile([128, 128], BF16, tag="aoT",
                                       name="aoT_sb")
                if via == "pe":
                    # tail blocks: PE transpose + ACT evac beats the ~2.3us
                    # DMA xbar latency; the pst ring is idle by then
                    tp = pst.tile([128, 128], BF16, tag="st", name="tp_ps")
                    nc.tensor.transpose(tp, aoN[(p, b)], eye)
                    nc.scalar.copy(aoT[(p, b)], tp)
                else:
                    nc.sync.dma_start(aoT[(p, b)], aoN[(p, b)], transpose=True)

            stg_tiles = {}

            def do_proj(t, n, evac="dve", split=False):
                """out-projection for query block t, column half n: contract
                both pairs (256 channels) into one po, evacuate bf16, DMA
                the finished output out."""
                po = psq.tile([128, 512], F32, tag="psq", name="po_ps")
                for p in range(2):
                    nc.tensor.matmul(
                        po,
                        lhsT=aoT[(p, t)],
                        rhs=wout_sb[:, p, n * 512:(n + 1) * 512],
                        start=(p == 0), stop=(p == 1),
                    )
                if n == 0:
                    stg_tiles[t] = pstg.tile([128, D], BF16, tag="stg",
                                             name="stg_sb")
                stg = stg_tiles[t]
                if evac == "act":  # tail blocks: ACT is done with exp there
                    nc.scalar.copy(stg[:, n * 512:(n + 1) * 512], po)
                else:
                    nc.vector.tensor_copy(stg[:, n * 512:(n + 1) * 512], po)
                if split:  # tail: per-half DMA so the drain overlaps
                    nc.sync.dma_start(
                        out_d[t * 128:(t + 1) * 128, n * 512:(n + 1) * 512],
                        stg[:, n * 512:(n + 1) * 512])
                elif n == 1:
                    nc.sync.dma_start(out_d[t * 128:(t + 1) * 128, :], stg)

            # ---- schedule ----

            def chain_unit(p, b, via="dma"):
                def u():
                    do_chain(p, 0, b)
                    do_chain(p, 1, b)
                    do_transpose(p, b, via)
                # chains stream v tiles 0..b as matmul rhs
                return (u, tuple(f"v{i}" for i in range(b + 1)),
                        (b + 1) * 130)

            def stripe_units(p, j, i0=0, i1=None, chains=True, via="dma",
                             mask_eng="pool"):
                units = []
                for i in range(i0, 4 * j + 4 if i1 is None else i1):
                    r = max(0, 128 * i - 512 * j)
                    units.append(
                        (lambda p=p, i=i, j=j, me=mask_eng:
                         do_st(p, i, j, me),
                         (f"m{2 + p}n{i // 4}", f"m{p}n{j}"),
                         2 * (512 - r)))
                    if chains and i >= 4 * j:
                        units.append(chain_unit(p, i, via))
                return units

            def merge(u1, u2):
                """cost-proportional merge of two unit lists"""
                c1 = sum(u[2] for u in u1)
                c2 = sum(u[2] for u in u2)
                out = []
                i1 = i2 = 0
                a1 = a2 = 0
                while i1 < len(u1) or i2 < len(u2):
                    if i2 >= len(u2) or (i1 < len(u1) and a1 * c2 <= a2 * c1):
                        out.append(u1[i1])
                        a1 += u1[i1][2]
                        i1 += 1
                    else:
                        out.append(u2[i2])
                        a2 += u2[i2][2]
                        i2 += 1
                return out

            emitted = set()

            def interleave(primary, filler):
                """primary: [(unit, needs, cost)]; filler:
                [(name, unit, deps, cost)]. Emits fillers in cost proportion
                to the primaries, but pulls a named filler (and its deps,
                recursively) forward whenever an upcoming primary depends on
                it, so every engine stream stays dependency-ordered (engines
                execute in order; a consumer emitted before its producer
                would deadlock real hardware)."""
                fi = 0
                index = {f[0]: f for f in filler}
                fired_cost = 0
                pcost_total = sum(u[2] for u in primary)
                fcost_total = sum(f[3] for f in filler)

                def fire_name(nm):
                    nonlocal fired_cost
                    if nm in emitted:
                        return
                    assert nm in index, f"dependency {nm} not in filler list"
                    _, u, deps, cost = index[nm]
                    for d in deps:
                        fire_name(d)
                    emitted.add(nm)
                    fired_cost += cost
                    u()

                pcost = 0
                for pu, needs, cost in primary:
                    for nm in needs:
                        if nm not in emitted:
                            fire_name(nm)
                    pu()
                    pcost += cost
                    while (fi < len(filler)
                           and fired_cost * pcost_total
                           < fcost_total * pcost):
                        fire_name(filler[fi][0])
                        fi += 1
                while fi < len(filler):
                    fire_name(filler[fi][0])
                    fi += 1

            def qk_u(m, n):
                """two filler units: matmul halves; name completes at part 1"""
                return [(f"m{m}n{n}_a", lambda: do_qk(m, n, 0), (), 2048),
                        (f"m{m}n{n}", lambda: do_qk(m, n, 1),
                         (f"m{m}n{n}_a",), 2048)]

            def v_u(t):
                return [(f"v{t}_a", lambda: do_v(t, 0), (), 1024),
                        (f"v{t}", lambda: do_v(t, 1), (f"v{t}_a",), 1024)]

            def proj_u(t, n, evac="dve"):
                return [(f"proj{t}_{n}", lambda: do_proj(t, n, evac), (),
                         1024)]

            # preamble: pair-0 q/k first column chunks, kc-interleaved so
            # both accumulators advance as each xt/wqk chunk lands
            ps_a = psq.tile([128, 512], F32, tag="psq", name="ps_qk")
            ps_b = psq.tile([128, 512], F32, tag="psq", name="ps_qk")
            for kc in range(KT):
                for m, ps in ((0, ps_a), (2, ps_b)):
                    nc.tensor.matmul(
                        ps,
                        lhsT=wqk_sb[:, kc, m * 128:(m + 1) * 128],
                        rhs=xt_sb[:, kc, 0:512],
                        start=(kc == 0), stop=(kc == KT - 1),
                    )
            nc.vector.tensor_mul(qp[0][:, 0:512], ps_a, gate_sb[:, 0, 0:512])
            nc.vector.tensor_copy(kp[0][:, 0:512], ps_b)
            emitted.update({"m0n0", "m2n0"})

            # Engines execute their streams in order, so every unit is
            # emitted after everything it depends on: v tiles land before the
            # chains that read them, q/k chunks a super-stripe before their
            # stripes, proj after both pairs' transposes. Pair 1 runs one
            # super-stripe behind pair 0; its stripe-3 scores/exp are
            # pre-computed inside ss3 so the final chain phase (ss4) has no
            # exp dependency and the drain stays short.
            def flat(*groups):
                return [f for g in groups for f in g]

            # ss0: p0 s0
            interleave(
                stripe_units(0, 0),
                flat(v_u(0), qk_u(3, 0), v_u(1), qk_u(2, 1), v_u(2),
                     qk_u(0, 1), v_u(3), qk_u(1, 0)),
            )
            # ss1: p0 s1 + p1 s0
            interleave(
                merge(stripe_units(0, 1), stripe_units(1, 0)),
                flat(qk_u(3, 1), v_u(4), qk_u(1, 1), v_u(5), qk_u(2, 2),
                     v_u(6), qk_u(0, 2), v_u(7)),
            )
            # ss2: p0 s2 + p1 s1 | combined proj t0-3 possible now
            interleave(
                merge(stripe_units(0, 2), stripe_units(1, 1)),
                flat(qk_u(3, 2), v_u(8), qk_u(1, 2), v_u(9), qk_u(2, 3),
                     v_u(10), qk_u(0, 3), v_u(11), qk_u(1, 3), qk_u(3, 3)),
            )
            # ss3: p0 s3 + p1 s2 + first part of p1 s3 scores | proj t0-7
            interleave(
                merge(merge(stripe_units(0, 3), stripe_units(1, 2)),
                      stripe_units(1, 3, i1=4)),
                flat(v_u(12), v_u(13), v_u(14), v_u(15),
                     *[proj_u(t, n) for t in range(0, 8) for n in range(2)]),
            )
            # ss4: rest of p1 s3 + its chains (PE transpose, DVE masks)
            interleave(
                stripe_units(1, 3, i0=4, via="pe", mask_eng="dve"),
                flat(*[proj_u(t, n) for t in range(8, 12) for n in range(2)]),
            )
            # tail: proj t12-15, evacuation split across DVE and ACT,
            # per-half output DMA
            for t in range(12, 16):
                for n in range(2):
                    do_proj(t, n, "act" if n else "dve", split=True)
    nc.compile()
    return nc


def _prep_inputs(x, w_qkv, w_out, bandha_gate):
    bf = ml_dtypes.bfloat16
    t = np.arange(T)
    gate_full = np.empty((16, T), np.float64)
    for h in range(16):
        cyc = TALA[h % len(TALA)]
        gate_full[h] = 1.0 / (
            1.0 + np.exp(-bandha_gate[h, t % cyc].astype(np.float64)))
    tri = (np.arange(128)[None, :] >= np.arange(128)[:, None]).astype(bf)
    eye = np.eye(128, dtype=np.float32).astype(bf)

    in_maps = []
    for c in range(8):
        b, g = c // 4, c % 4
        xt = np.ascontiguousarray(x[b].T).astype(bf)
        # m-tiles: q-pair0, q-pair1, k-pair0, k-pair1 (128 channels each)
        q0 = w_qkv[:, g * 256:g * 256 + 128]
        q1 = w_qkv[:, g * 256 + 128:g * 256 + 256]
        k0 = w_qkv[:, D + g * 256:D + g * 256 + 128]
        k1 = w_qkv[:, D + g * 256 + 128:D + g * 256 + 256]
        wqk = np.concatenate([q0, q1, k0, k1], axis=1).astype(bf)
        wv = np.ascontiguousarray(
            w_qkv[:, 2 * D + g * 256:2 * D + (g + 1) * 256]).astype(bf)
        # wout rows: within-pair channel, cols: (pair, d)
        wo = np.stack([w_out[g * 256:g * 256 + 128, :],
                       w_out[g * 256 + 128:g * 256 + 256, :]], axis=1)
        wo = np.ascontiguousarray(wo.reshape(128, 2 * D)).astype(bf)
        # gate rows: (pair, within-pair channel); pair p = heads {2p, 2p+1}
        gb = np.repeat(gate_full[4 * g:4 * g + 4].astype(np.float32),
                       HD, axis=0).astype(bf)
        in_maps.append({"xt": xt, "wqk": wqk, "wv": wv, "wout": wo,
                        "gate": np.ascontiguousarray(gb), "tri": tri,
                        "eye": eye})
    return in_maps


def kernel(**inputs):
    global LAST
    x = np.asarray(inputs["x"], np.float32)
    w_qkv = np.asarray(inputs["w_qkv"], np.float32)
    w_out = np.asarray(inputs["w_out"], np.float32)
    bandha_gate = np.asarray(inputs["bandha_gate"], np.float32)

    in_maps = _prep_inputs(x, w_qkv, w_out, bandha_gate)
    nc = build_nc()
    res = run_bass_kernel_spmd(
        nc, in_maps, core_ids=list(range(8)),
        trace=os.environ.get("BANDHA_TRACE") == "1",
    )
    LAST = res
    full = np.empty((2, T, D), np.float32)
    for b in range(2):
        acc = np.zeros((T, D), np.float32)
        for g in range(4):
            acc += np.asarray(res.results[4 * b + g]["out"],
                              dtype=np.float32)
        full[b] = acc
    return full
